# revision 1
# baseline (speedup 1.0000x reference)
"""Trainium2 Bass kernel for a CQT (constant-Q transform) nn.Module.

Reference computation (per batch sample b, channel c):
    out[b, c, k, f, 0] = sum_t x[b, c, f*HOP + t] * w_re[k, t]
    out[b, c, k, f, 1] = sum_t x[b, c, f*HOP + t] * w_im[k, t]
where w_re/w_im are Hann-windowed complex exponentials with per-bin ragged
lengths (longest 11340 samples), HOP=512, 84 bins, 409 frames.

Strategy: data-parallel over the batch (1 sample per NeuronCore, 8 cores).
Per core the correlation is a banded matmul: the contraction axis t is split
into 89 chunks of 128; chunk c needs x samples x[(f + c//4)*512 + (c%4)*128 + r].
The signal is laid out once in SBUF as Xt[r, ch, rc, m] = x[ch, m*512+rc*128+r]
so every chunk's moving operand is a 410-column slice of a resident tile
(410 = 409 frames padded to the even count fp32r requires).

Weight rows are interleaved (re_k, im_k) pairs sorted by descending window
length, so the rows active in a chunk are always a prefix. Rows 0..127
(bins 0..63) form accumulation group G1 (89 chunks); rows 128..167
(bins 64..83, windows <= 281 samples) form group G2 (3 chunks). Weights are
stored column-compacted (only active rows per chunk): 1.2 MB of DMA instead
of 7.9 MB, with no change in matmul cost.

Mixed precision + PE column tiling: chunks 0..14 (>64 active rows) run
serially in float32r (full fp32 data at 1 cycle/row). The ragged tail is
where most chunks live but few rows are active, so those matmuls are packed
into disjoint column strips of the 128x128 PE array with `tile_position` and
run concurrently: chunks 15..35 (<=64 rows) two per pass, chunks 36..88
(<=32 rows) four per pass, each strip accumulating into its own PSUM band;
a cheap DVE reduction folds the bands at the end. fp32r forbids nonzero PSUM
start partitions, so the tail runs in fp16 (same 1 cycle/row; windows and
white-noise signal are well inside fp16 range). Measured end-to-end relative
error vs the fp32 reference: ~2.3e-4.
"""

import math
import os as _os
from contextlib import ExitStack

import numpy as np

import concourse.bass as bass
import concourse.mybir as mybir
import concourse.tile as tile
from concourse import bacc
from concourse.bass_utils import run_bass_kernel_spmd

# ---- problem constants (hardcoded CQT spec) ----
SR = 22050
N_BINS = 84
BPO = 12
FMIN = 32.7
HOP = 512
B, C, T = 8, 2, 220500
N_CORES = 8

LMAX = 11340           # longest window
F = 409                # frames: 1 + (T - LMAX)//HOP
NCHUNK = 89            # ceil(LMAX/128) contraction chunks
MBLK = 432             # 512-sample blocks of x: (F-1)+(NCHUNK-1)//4+1 = 431, +1 pad
FP = 410               # fp32r needs an even moving free dim; frame 409 is junk
NROWS = 2 * N_BINS     # interleaved (re, im) weight rows
G1ROWS = 128           # group 1 = rows 0..127  (bins 0..63)
G2ROWS = NROWS - G1ROWS  # 40 rows (bins 64..83)
HB_S = 6               # first head chunk with <=96 rows: fp16, carries a
                       # 1-strip partner at PE column position 96
C2S = 14               # first chunk with <=64 active rows (2-way col tiling)
C4S = 36               # first chunk with <=32 active rows (4-way col tiling)
N_WARM = int(_os.environ.get("K_NWARM", "5"))  # PE warm-up matmuls

MM_DT = mybir.dt.float32r   # head matmul dtype (full-rate fp32)
TL_DT = mybir.dt.float16    # tail matmul dtype (col-tiling legal, 1 cyc/row)

_PREP = None
_NC = None
LAST_RESULTS = None


def _params():
    """Host-side constants: compacted weight arrays + chunk geometry."""
    global _PREP
    if _PREP is not None:
        return _PREP

    Q = 1.0 / (2.0 ** (1.0 / BPO) - 1.0)
    freqs = FMIN * 2.0 ** (np.arange(N_BINS, dtype=np.float64) / BPO)
    lengths = np.round(Q * SR / freqs).astype(np.int64)
    assert int(lengths.max()) == LMAX

    t = np.arange(LMAX, dtype=np.float64)
    L = lengths.astype(np.float64)[:, None]
    mask = (t[None, :] < L).astype(np.float64)
    win = 0.5 * (1.0 - np.cos(2.0 * math.pi * t[None, :] / L)) * mask
    phase = (2.0 * math.pi / SR) * freqs[:, None] * t[None, :]
    w_re = (win * np.cos(phase)).astype(np.float32)
    w_im = (-win * np.sin(phase)).astype(np.float32)

    # rows 2k / 2k+1 = re_k / im_k; zero-pad time to NCHUNK*128
    W = np.zeros((NROWS, NCHUNK * 128), dtype=np.float32)
    W[0::2, :LMAX] = w_re
    W[1::2, :LMAX] = w_im
    WT = np.ascontiguousarray(W.T)  # (NCHUNK*128, NROWS)

    n_act = np.array([(lengths > 128 * c).sum() for c in range(NCHUNK)])
    assert n_act[0] == N_BINS and n_act[-1] >= 1
    mG1 = np.minimum(G1ROWS, 2 * n_act).astype(np.int64)
    assert mG1[C2S] <= 64 and mG1[C2S - 1] > 64
    assert mG1[C4S] <= 32 and mG1[C4S - 1] > 32
    G2C = math.ceil(int(lengths[G1ROWS // 2]) / 128)  # chunks needed by bin 64
    mG2 = (2 * n_act[:G2C] - G1ROWS).astype(np.int64)
    assert mG2[0] == G2ROWS and (mG2 > 0).all()

    base = np.zeros(NCHUNK + 1, dtype=np.int64)
    base[1:] = np.cumsum(mG1)
    SG1 = int(base[-1])
    g2base = np.zeros(G2C + 1, dtype=np.int64)
    g2base[1:] = np.cumsum(mG2)
    SG2 = int(g2base[-1])

    wg1 = np.zeros((128, SG1), dtype=np.float32)
    for c in range(NCHUNK):
        wg1[:, base[c]:base[c + 1]] = WT[128 * c:128 * (c + 1), :mG1[c]]
    wg2 = np.zeros((128, SG2), dtype=np.float32)
    for c in range(G2C):
        wg2[:, g2base[c]:g2base[c + 1]] = WT[128 * c:128 * (c + 1),
                                             G1ROWS:G1ROWS + mG2[c]]

    SH = int(base[HB_S])         # fp32r head weight columns (chunks 0..5)
    SHB = int(base[C2S]) - SH    # fp16 paired-head columns (chunks 6..13)
    ST = SG1 - SH - SHB          # fp16 tail columns (chunks 14..88)
    wg1h = np.ascontiguousarray(wg1[:, :SH])
    wg1hb = np.ascontiguousarray(wg1[:, SH:SH + SHB]).astype(np.float16)
    wg1t = np.ascontiguousarray(wg1[:, SH + SHB:]).astype(np.float16)

    _PREP = dict(mG1=mG1, mG2=mG2, G2C=G2C, base=base, g2base=g2base,
                 SH=SH, SHB=SHB, ST=ST, SG2=SG2,
                 wg1h=wg1h, wg1hb=wg1hb, wg1t=wg1t, wg2=wg2)
    return _PREP


def _build_nc(rep=1):
    """Build the per-core Bass module. rep>1 wraps the matmul streams in a
    For_i hardware loop (benchmarking only)."""
    p = _params()
    mG1, mG2, G2C = p["mG1"], p["mG2"], p["G2C"]
    base, g2base = p["base"], p["g2base"]
    SH, SHB, ST, SG2 = p["SH"], p["SHB"], p["ST"], p["SG2"]

    nc = bacc.Bacc(None, target_bir_lowering=False)
    xt_d = nc.dram_tensor("xt", (C, 4, 128, MBLK), MM_DT, kind="ExternalInput")
    xtb_d = nc.dram_tensor("xtb", (C, 4, 128, MBLK), TL_DT,
                           kind="ExternalInput")
    wh_d = nc.dram_tensor("wh", (128, SH), MM_DT, kind="ExternalInput")
    whb_d = nc.dram_tensor("whb", (128, SHB), TL_DT, kind="ExternalInput")
    wt_d = nc.dram_tensor("wt", (128, ST), TL_DT, kind="ExternalInput")
    wg2_d = nc.dram_tensor("wg2", (128, SG2), TL_DT, kind="ExternalInput")
    out_d = nc.dram_tensor("out", (C, NROWS, F), mybir.dt.float32,
                           kind="ExternalOutput")

    with ExitStack() as ctx:
        tc = ctx.enter_context(tile.TileContext(nc))
        xp = ctx.enter_context(tc.tile_pool(name="xp", bufs=1))
        wp = ctx.enter_context(tc.tile_pool(name="wp", bufs=1))
        op = ctx.enter_context(tc.tile_pool(name="op", bufs=1))
        pp = ctx.enter_context(tc.tile_pool(name="pp", bufs=1, space="PSUM"))

        # PSUM: 4 banks per channel (head, G2, 2-way bands, 4-way bands)
        # full-bank width (512) so partition-sliced band APs stay bank-local
        ps1 = {ch: pp.tile([128, 512], mybir.dt.float32, name=f"ps1_{ch}",
                           tag=f"ps1_{ch}") for ch in range(C)}
        ps2 = {ch: pp.tile([128, 512], mybir.dt.float32, name=f"ps2_{ch}",
                           tag=f"ps2_{ch}") for ch in range(C)}
        pt2 = {ch: pp.tile([128, 512], mybir.dt.float32, name=f"pt2_{ch}",
                           tag=f"pt2_{ch}") for ch in range(C)}
        pt4 = {ch: pp.tile([128, 512], mybir.dt.float32, name=f"pt4_{ch}",
                           tag=f"pt4_{ch}") for ch in range(C)}

        # PE warm-up: dummy matmuls on a memset scratch tile bridge the cold
        # pstate while input DMAs run; target ch1's 4-way bank, whose real
        # use starts much later with start=True (pending-zero overwrites).
        warm_sb = xp.tile([128, 128], MM_DT, name="warm_sb", tag="warm_sb")
        nc.vector.memset(warm_sb[:].bitcast(mybir.dt.float32), 0.0)
        for _ in range(N_WARM):
            nc.tensor.matmul(pt4[1][:, 0:128], warm_sb[:, :], warm_sb[:, :],
                             start=True, stop=True, skip_group_check=True)

        # --- SBUF tiles + input DMA plan (two parallel queues) ---
        wh_sb = wp.tile([128, SH], MM_DT, name="wh_sb", tag="wh_sb")
        whb_sb = wp.tile([128, SHB], TL_DT, name="whb_sb", tag="whb_sb")
        wt_sb = wp.tile([128, ST], TL_DT, name="wt_sb", tag="wt_sb")
        wg2_sb = wp.tile([128, SG2], TL_DT, name="wg2_sb", tag="wg2_sb")
        xt_sb = {(ch, rc): xp.tile([128, MBLK], MM_DT, name=f"x_{ch}_{rc}",
                                   tag=f"x_{ch}_{rc}")
                 for ch in range(C) for rc in range(4)}
        xtb_sb = {(ch, rc): xp.tile([128, MBLK], TL_DT, name=f"xb_{ch}_{rc}",
                                    tag=f"xb_{ch}_{rc}")
                  for ch in range(C) for rc in range(4)}

        # gpsimd (SWDGE) stream: weights in consumption order. Split the
        # head weights so the first matmul is gated by a single small block.
        nc.gpsimd.dma_start(wh_sb[:, 0:int(base[1])], wh_d[:, 0:int(base[1])])
        nc.gpsimd.dma_start(wh_sb[:, int(base[1]):SH],
                            wh_d[:, int(base[1]):SH])
        nc.gpsimd.dma_start(whb_sb[:], whb_d[:])
        half_t = (ST // 2) & ~1
        nc.gpsimd.dma_start(wt_sb[:, 0:half_t], wt_d[:, 0:half_t])
        # sync (HWDGE) stream: ch0 signal (fp32r then fp16), G2 weights,
        # second half of the fp16 tail weights (hedges SWDGE bandwidth),
        # then ch1 (needed only from halfway).
        nc.sync.dma_start(xt_sb[0, 0][:], xt_d[0, 0])
        nc.sync.dma_start(wg2_sb[:], wg2_d[:])
        for rc in range(1, 4):
            nc.sync.dma_start(xt_sb[0, rc][:], xt_d[0, rc])
        for rc in (2, 3, 0, 1):  # first 2-way chunks consume rc 2,3 first
            nc.sync.dma_start(xtb_sb[0, rc][:], xtb_d[0, rc])
        nc.sync.dma_start(wt_sb[:, half_t:ST], wt_d[:, half_t:ST])
        for rc in range(4):
            nc.sync.dma_start(xt_sb[1, rc][:], xt_d[1, rc])
        for rc in (2, 3, 0, 1):
            nc.sync.dma_start(xtb_sb[1, rc][:], xtb_d[1, rc])

        # --- fp16 band plan (per channel, identical both channels) ---
        # A band = (psum tile, partition position): an independent strip
        # accumulator. Head chunks 6..13 (fp16, <=96 rows, strips 0-2) each
        # carry a 1-strip pt4@96 partner. G2's three chunks ride in hybrid
        # passes (G2c0@ps2:0 + 2-way partner; G2c1@ps2:0 + G2c2@ps2:64 +
        # two pt4 partners — G2c2's partitions must not overlap G2c0/1's).
        # Chunks are dealt to bands from per-class iterators in emission
        # order, which keeps every band's M sequence descending (so its
        # first, start=True matmul pending-zero-arms all rows it ever uses).
        # `stop` is inert under skip_group_check (and a no-op on HW).
        def g1_chunk(c, cls):
            j, rc = divmod(c, 4)
            offs = {"hb": SH, "tw": SH + SHB, "fw": SH + SHB}
            return dict(m=int(mG1[c]), off=int(base[c]) - offs[cls],
                        j=j, rc=rc, cls=cls)

        def g2_chunk(c):
            j, rc = divmod(c, 4)
            return dict(m=int(mG2[c]), off=int(g2base[c]), j=j, rc=rc,
                        cls="g2")

        srcs = {
            "tw": [g1_chunk(c, "tw") for c in range(C2S, C4S)],    # 22, M<=64
            "fw": [g1_chunk(c, "fw") for c in range(C4S, NCHUNK)], # 53, M<=32
            "g2": [g2_chunk(c) for c in range(G2C)],               # 3
        }
        bands = {
            "hb":   dict(tile="ps1", pos=0,  cls="hb"),
            "2w0":  dict(tile="pt2", pos=0,  cls="tw"),
            "2w64": dict(tile="pt2", pos=64, cls="tw"),
            "g2a":  dict(tile="ps2", pos=0,  cls="g2"),
            "g2b":  dict(tile="ps2", pos=64, cls="g2"),
            "4w0":  dict(tile="pt4", pos=0,  cls="fw"),
            "4w32": dict(tile="pt4", pos=32, cls="fw"),
            "4w64": dict(tile="pt4", pos=64, cls="fw"),
            "4w96": dict(tile="pt4", pos=96, cls="fw"),
        }
        order = (["g2a", "2w64"]                         # hybrid pass A
                 + ["g2a", "g2b", "4w32", "4w96"]        # hybrid pass B
                 + ["2w0", "2w64"] * 11
                 + ["4w0", "4w32", "4w64", "4w96"] * 13)
        band_rows = {}  # band -> rows its first (largest-M) chunk wrote

        def emit_streams():
            for ch in range(C):
                # fp32r head: chunks 0..5 serial, full array width.
                # start=True on chunk 0 (m=128) zero-arms the whole bank
                # region; later ragged prefixes accumulate (group checker
                # can't express this — skipped).
                for c in range(0, C2S):
                    j, rc = divmod(c, 4)
                    m = int(mG1[c])
                    if c < HB_S:
                        wsb_h = wh_sb[:, int(base[c]):int(base[c]) + m]
                        rhs_h = xt_sb[ch, rc][:, j:j + FP]
                    else:  # chunks 6..13 serial too, fp16 operands
                        o = int(base[c]) - SH
                        wsb_h = whb_sb[:, o:o + m]
                        rhs_h = xtb_sb[ch, rc][:, j:j + FP]
                    nc.tensor.matmul(
                        ps1[ch][0:m, 0:FP], wsb_h, rhs_h,
                        start=(c == 0), stop=False,
                        skip_group_check=True)
                # fp16 col-tiled section
                tiles = {"ps1": ps1[ch], "pt2": pt2[ch], "pt4": pt4[ch],
                         "ps2": ps2[ch]}
                wsbs = {"hb": whb_sb, "tw": wt_sb, "fw": wt_sb, "g2": wg2_sb}
                iters = {k: iter(q) for k, q in srcs.items()}
                first = {k: True for k in bands}
                for key in order:
                    b = bands[key]
                    cinfo = next(iters[b["cls"]], None)
                    if cinfo is None:
                        continue
                    m, pos = cinfo["m"], b["pos"]
                    if first[key]:
                        band_rows[key] = m
                    rhs = xtb_sb[ch, cinfo["rc"]][:,
                                                  cinfo["j"]:cinfo["j"] + FP]
                    wsb = wsbs[cinfo["cls"]]
                    off = cinfo["off"]
                    if key == "hb":
                        # 65..96-row chunk: a 3-strip col_grp mask (0x7) is
                        # not a legal ISA combination, so split at row 64 —
                        # strips 0-1 (0x3) + strip 2 (0x4) — both landing in
                        # ps1's own partitions; never re-arm ps1 (chunk 0
                        # started it).
                        nc.tensor.matmul(
                            ps1[ch][0:64, 0:FP], wsb[:, off:off + 64], rhs,
                            start=False, stop=True,
                            tile_position=(0, 0), skip_group_check=True)
                        nc.tensor.matmul(
                            ps1[ch][64:m, 0:FP], wsb[:, off + 64:off + m],
                            rhs, start=False, stop=True,
                            tile_position=(0, 64), skip_group_check=True)
                    else:
                        nc.tensor.matmul(
                            tiles[b["tile"]][pos:pos + m, 0:FP],
                            wsb[:, off:off + m], rhs,
                            start=first[key], stop=True,
                            tile_position=(0, pos),
                            skip_group_check=True)
                    first[key] = False
                for k, it in iters.items():
                    assert next(it, None) is None, f"class {k} not drained"

        if rep > 1:
            with tc.For_i(0, rep, 1) as _i:
                emit_streams()
        else:
            emit_streams()

        # fold the tail bands into the head accumulator and write out
        for ch in range(C):
            o1 = op.tile([128, F], mybir.dt.float32, name=f"o1_{ch}",
                         tag=f"o1_{ch}")
            o2 = op.tile([G2ROWS, F], mybir.dt.float32, name=f"o2_{ch}",
                         tag=f"o2_{ch}")
            # Emit chains in dependency order so the scheduler hoists them
            # into the matmul stream: rows 64..127 of G1 are final when the
            # fp32r head stops; ps2/pt2 bands stop mid-stream; only the four
            # pt4 adds wait for the last matmul.
            nc.vector.tensor_copy(o1[64:128, :], ps1[ch][64:128, 0:F])
            nc.sync.dma_start(out_d[ch, 64:G1ROWS, :], o1[64:128, :])
            nc.vector.tensor_copy(o2[:], ps2[ch][0:G2ROWS, 0:F])
            m2 = int(mG2[2])
            nc.vector.tensor_add(o2[0:m2, :], o2[0:m2, :],
                                 ps2[ch][64:64 + m2, 0:F])
            nc.sync.dma_start(out_d[ch, G1ROWS:NROWS, :], o2[:])
            nc.vector.tensor_copy(o1[0:64, :], ps1[ch][0:64, 0:F])
            for key in ("2w0", "2w64", "4w0", "4w32", "4w64", "4w96"):
                m = int(band_rows[key])  # rows this band ever wrote (<=64)
                pos = bands[key]["pos"]
                tl = {"pt2": pt2[ch], "pt4": pt4[ch]}[bands[key]["tile"]]
                nc.vector.tensor_add(o1[0:m, :], o1[0:m, :],
                                     tl[pos:pos + m, 0:F])
            nc.sync.dma_start(out_d[ch, 0:64, :], o1[0:64, :])
    nc.finalize()
    return nc


def get_nc():
    global _NC
    if _NC is None:
        _NC = _build_nc()
    return _NC


def _pack_x(xb):
    """(C, T) -> (C, 4, 128, MBLK) with xt[ch, rc, r, m] = x[ch, m*512+rc*128+r]."""
    xpad = np.zeros((C, MBLK * 512), dtype=np.float32)
    xpad[:, :T] = xb
    return np.ascontiguousarray(
        xpad.reshape(C, MBLK, 4, 128).transpose(0, 2, 3, 1))


def kernel(x):
    global LAST_RESULTS
    x = np.asarray(x, dtype=np.float32)
    assert x.shape == (B, C, T)
    p = _params()
    in_maps = []
    for b in range(B):
        xt = _pack_x(x[b])
        in_maps.append({"xt": xt, "xtb": xt.astype(np.float16),
                        "wh": p["wg1h"], "whb": p["wg1hb"], "wt": p["wg1t"],
                        "wg2": p["wg2"].astype(np.float16)})
    nc = get_nc()
    res = run_bass_kernel_spmd(nc, in_maps, core_ids=list(range(N_CORES)))
    LAST_RESULTS = res
    out = np.empty((B, C, N_BINS, F, 2), dtype=np.float32)
    for b in range(B):
        raw = np.asarray(res.results[b]["out"])  # (C, NROWS, F)
        out[b] = raw.reshape(C, N_BINS, 2, F).transpose(0, 1, 3, 2)
    return out



# revision 3
# speedup vs baseline: 3.3156x; 3.3156x over previous
"""Trainium2 Bass kernel for a CQT (constant-Q transform) nn.Module.

Reference computation (per batch sample b, channel c):
    out[b, c, k, f, 0] = sum_t x[b, c, f*HOP + t] * w_re[k, t]
    out[b, c, k, f, 1] = sum_t x[b, c, f*HOP + t] * w_im[k, t]
where w_re/w_im are Hann-windowed complex exponentials with per-bin ragged
lengths (longest 11340 samples), HOP=512, 84 bins, 409 frames.

Strategy: data-parallel over the batch (1 sample per NeuronCore, 8 cores).
Per core the correlation is factored through a two-level block basis, which
cuts the tensor-engine moving-row count ~6x vs the direct banded matmul:

  1. Stage 1: each 512-sample hop block of x is projected onto a rank-128
     orthonormal basis B0 (SVD of all 512-aligned weight-window segments;
     numerical rank ~112, tail 5e-15).  4 matmuls x 436 moving rows.
  2. Stage 2: coefficients of 4 adjacent blocks are combined into 2048-block
     coefficients in basis B2 (SVD of the 2048-aligned segments of bins
     0..53; rank 87 @1e-8).  C_j = B0 @ B2_piece_j^T.  4 matmuls x 432.
  3. Final: bins 54..83 (window <= 512 samples, a single hop block)
     correlate against Y0 directly (1 matmul x 410); bins 0..53 correlate
     against Y2 at 6 block shifts (6 matmuls x 410, ragged row prefixes
     accumulating in one PSUM bank).

Per channel that is 6342 moving rows (vs 37720 direct); the whole basis +
coefficient payload is ~340 KB vs 7.9 MB of raw windows.  Everything runs
in fp16 (measured end-to-end relative error vs the fp32 reference ~5e-4);
output is written fp16 and widened on the host.
"""

import math
import os as _os
from contextlib import ExitStack

import numpy as np

import concourse.bass as bass
import concourse.mybir as mybir
import concourse.tile as tile
from concourse import bacc
from concourse.bass_utils import run_bass_kernel_spmd

# ---- problem constants (hardcoded CQT spec) ----
SR = 22050
N_BINS = 84
BPO = 12
FMIN = 32.7
HOP = 512
B, C, T = 8, 2, 220500
N_CORES = 8

LMAX = 11340           # longest window
F = 409                # frames: 1 + (T - LMAX)//HOP
FP = 410               # even moving-dim padding for the final stage
NB0 = 436              # 512-sample x blocks incl. pad (431 real + shift room)
NB2 = 432              # 2048-block coefficient positions (F + 4*5 + pad)
KL2 = 54               # bins 0..KL2-1 via 2048-blocks, rest via 512-blocks
P0 = 128               # level-0 basis size
Q2 = 128               # level-2 basis size
NROWS = 2 * N_BINS
RA = NROWS - 2 * KL2   # 60 rows (bins 54..83) through the level-0 path
N_WARM = int(_os.environ.get("K_NWARM", "8"))
WARM_MV = 256          # moving rows per warm-up matmul

TL_DT = mybir.dt.float16

_PREP = None
_NC = None
LAST_RESULTS = None


def _params():
    """Host-side constants: bases + projection coefficients (float64 SVD)."""
    global _PREP
    if _PREP is not None:
        return _PREP

    Qf = 1.0 / (2.0 ** (1.0 / BPO) - 1.0)
    freqs = FMIN * 2.0 ** (np.arange(N_BINS, dtype=np.float64) / BPO)
    lengths = np.round(Qf * SR / freqs).astype(np.int64)
    assert int(lengths.max()) == LMAX

    t = np.arange(LMAX, dtype=np.float64)
    L = lengths.astype(np.float64)[:, None]
    mask = (t[None, :] < L).astype(np.float64)
    win = 0.5 * (1.0 - np.cos(2.0 * math.pi * t[None, :] / L)) * mask
    phase = (2.0 * math.pi / SR) * freqs[:, None] * t[None, :]
    W = np.zeros((NROWS, 6 * 2048), dtype=np.float64)
    W[0::2, :LMAX] = win * np.cos(phase)
    W[1::2, :LMAX] = -win * np.sin(phase)

    # B0: orthonormal basis of all 512-aligned window segments
    segs0 = []
    for k in range(N_BINS):
        for u in range(math.ceil(int(lengths[k]) / 512)):
            segs0.append(W[2 * k, 512 * u:512 * (u + 1)])
            segs0.append(W[2 * k + 1, 512 * u:512 * (u + 1)])
    _, _, V0 = np.linalg.svd(np.array(segs0), full_matrices=False)
    B0 = V0[:P0]                                    # (128, 512)

    # B2: basis of the 2048-aligned segments of bins 0..KL2-1
    segs2 = []
    for k in range(KL2):
        for u in range(math.ceil(int(lengths[k]) / 2048)):
            segs2.append(W[2 * k, 2048 * u:2048 * (u + 1)])
            segs2.append(W[2 * k + 1, 2048 * u:2048 * (u + 1)])
    _, _, V2 = np.linalg.svd(np.array(segs2), full_matrices=False)
    B2 = V2[:Q2]                                    # (128, 2048)

    # stage-2 combiners and final-stage projection coefficients
    Cj = np.stack([B0 @ B2[:, 512 * j:512 * (j + 1)].T
                   for j in range(4)])              # (4, 128, 128)
    A0 = B0 @ W[2 * KL2:, :512].T                   # (128, RA)
    nb2 = np.ceil(lengths[:KL2] / 2048.0).astype(np.int64)
    U2 = int(nb2.max())                             # 6
    rows_u2 = [2 * int((nb2 > u).sum()) for u in range(U2)]
    A2 = [B2 @ W[:rows_u2[u], 2048 * u:2048 * (u + 1)].T
          for u in range(U2)]                       # (128, rows_u2[u]) each

    # device weight tensors (fp16)
    # wb0[r, 128*rc + p] = B0[p, 128*rc + r]  (stationary for stage-1 chunk rc)
    wb0 = np.zeros((128, 512), dtype=np.float16)
    for rc in range(4):
        wb0[:, 128 * rc:128 * (rc + 1)] = B0[:, 128 * rc:128 * (rc + 1)].T
    # wrest = [C_0..C_3 | A0 | A2 compact]
    a2base = np.zeros(U2 + 1, dtype=np.int64)
    a2base[1:] = np.cumsum(rows_u2)
    SA2 = int(a2base[-1])
    wrest = np.zeros((128, 512 + RA + SA2), dtype=np.float16)
    for j in range(4):
        wrest[:, 128 * j:128 * (j + 1)] = Cj[j]
    wrest[:, 512:512 + RA] = A0
    for u in range(U2):
        wrest[:, 512 + RA + int(a2base[u]):512 + RA + int(a2base[u + 1])] = \
            A2[u]

    _PREP = dict(wb0=wb0, wrest=wrest, rows_u2=rows_u2, a2base=a2base,
                 U2=U2, SA2=SA2)
    return _PREP


def _build_nc():
    p = _params()
    rows_u2, a2base, U2, SA2 = (p["rows_u2"], p["a2base"], p["U2"], p["SA2"])
    WREST = 512 + RA + SA2

    nc = bacc.Bacc(None, target_bir_lowering=False)
    xtb_d = nc.dram_tensor("xtb", (C, 4, 128, NB0), TL_DT,
                           kind="ExternalInput")
    wb0_d = nc.dram_tensor("wb0", (128, 512), TL_DT, kind="ExternalInput")
    wrest_d = nc.dram_tensor("wrest", (128, WREST), TL_DT,
                             kind="ExternalInput")
    out_d = nc.dram_tensor("out", (C, NROWS, F), TL_DT, kind="ExternalOutput")

    with ExitStack() as ctx:
        tc = ctx.enter_context(tile.TileContext(nc))
        xp = ctx.enter_context(tc.tile_pool(name="xp", bufs=1))
        wp = ctx.enter_context(tc.tile_pool(name="wp", bufs=1))
        yp = ctx.enter_context(tc.tile_pool(name="yp", bufs=1))
        op = ctx.enter_context(tc.tile_pool(name="op", bufs=1))
        pp = ctx.enter_context(tc.tile_pool(name="pp", bufs=1, space="PSUM"))

        # PSUM: 4 banks per channel (Y0, Y2, outA, outB) = all 8 banks
        y0_ps = {ch: pp.tile([128, 512], mybir.dt.float32, name=f"y0p_{ch}",
                             tag=f"y0p_{ch}") for ch in range(C)}
        y2_ps = {ch: pp.tile([128, 512], mybir.dt.float32, name=f"y2p_{ch}",
                             tag=f"y2p_{ch}") for ch in range(C)}
        oa_ps = {ch: pp.tile([128, 512], mybir.dt.float32, name=f"oap_{ch}",
                             tag=f"oap_{ch}") for ch in range(C)}
        ob_ps = {ch: pp.tile([128, 512], mybir.dt.float32, name=f"obp_{ch}",
                             tag=f"obp_{ch}") for ch in range(C)}

        # PE warm-up: dummy matmuls bridge the p-state ramp while input DMAs
        # run; target ch1's outB bank, whose real use starts last and re-arms
        # with start=True.
        warm_sb = xp.tile([128, WARM_MV], TL_DT, name="warm_sb", tag="warm_sb")
        nc.vector.memset(warm_sb[:].bitcast(mybir.dt.float32), 0.0)
        for _ in range(N_WARM):
            nc.tensor.matmul(ob_ps[1][:, 0:WARM_MV], warm_sb[:, 0:128],
                             warm_sb[:, 0:WARM_MV],
                             start=True, stop=True, skip_group_check=True)

        # --- SBUF tiles ---
        wb0_sb = wp.tile([128, 512], TL_DT, name="wb0_sb", tag="wb0_sb")
        wrest_sb = wp.tile([128, WREST], TL_DT, name="wrest_sb",
                           tag="wrest_sb")
        xtb_sb = {(ch, rc): xp.tile([128, NB0], TL_DT, name=f"x_{ch}_{rc}",
                                    tag=f"x_{ch}_{rc}")
                  for ch in range(C) for rc in range(4)}
        y0_sb = {ch: yp.tile([128, NB0], TL_DT, name=f"y0_{ch}",
                             tag=f"y0_{ch}") for ch in range(C)}
        y2_sb = {ch: yp.tile([128, NB2], TL_DT, name=f"y2_{ch}",
                             tag=f"y2_{ch}") for ch in range(C)}

        # --- input DMA plan (two queues) ---
        # sync (HWDGE): ch0 signal + stage-1 basis, interleaved so the first
        # stage-1 matmul is gated by two small transfers.
        nc.sync.dma_start(wb0_sb[:], wb0_d[:])
        for rc in range(4):
            nc.sync.dma_start(xtb_sb[0, rc][:], xtb_d[0, rc])
        # gpsimd (SWDGE): ch1 signal, then the stage-2/final coefficients.
        for rc in range(4):
            nc.gpsimd.dma_start(xtb_sb[1, rc][:], xtb_d[1, rc])
        nc.gpsimd.dma_start(wrest_sb[:], wrest_d[:])

        # --- matmul stream ---
        def s1(ch):
            for rc in range(4):
                nc.tensor.matmul(y0_ps[ch][0:128, 0:NB0],
                                 wb0_sb[:, 128 * rc:128 * (rc + 1)],
                                 xtb_sb[ch, rc][:, 0:NB0],
                                 start=(rc == 0), stop=(rc == 3),
                                 skip_group_check=True)

        def s2(ch):
            for j in range(4):
                nc.tensor.matmul(y2_ps[ch][0:128, 0:NB2],
                                 wrest_sb[:, 128 * j:128 * (j + 1)],
                                 y0_sb[ch][:, j:j + NB2],
                                 start=(j == 0), stop=(j == 3),
                                 skip_group_check=True)

        def f0(ch):
            nc.tensor.matmul(oa_ps[ch][0:RA, 0:FP],
                             wrest_sb[:, 512:512 + RA],
                             y0_sb[ch][:, 0:FP],
                             start=True, stop=True, skip_group_check=True)

        def f2(ch):
            for u in range(U2):
                m = rows_u2[u]
                off = 512 + RA + int(a2base[u])
                nc.tensor.matmul(ob_ps[ch][0:m, 0:FP],
                                 wrest_sb[:, off:off + m],
                                 y2_sb[ch][:, 4 * u:4 * u + FP],
                                 start=(u == 0), stop=(u == U2 - 1),
                                 skip_group_check=True)

        # PE stream with the PSUM->SBUF casts (DVE) interleaved in
        # dependency order; the copies overlap the next PE group.
        s1(0)
        nc.vector.tensor_copy(y0_sb[0][:], y0_ps[0][:, 0:NB0])
        s1(1)
        nc.vector.tensor_copy(y0_sb[1][:], y0_ps[1][:, 0:NB0])
        s2(0)
        f0(0)
        nc.vector.tensor_copy(y2_sb[0][:], y2_ps[0][:, 0:NB2])
        s2(1)
        f0(1)
        nc.vector.tensor_copy(y2_sb[1][:], y2_ps[1][:, 0:NB2])
        f2(0)
        f2(1)

        # --- outputs: cast to fp16 and DMA per group ---
        for ch in range(C):
            oa = op.tile([RA, F], TL_DT, name=f"oa_{ch}", tag=f"oa_{ch}")
            ob = op.tile([108, F], TL_DT, name=f"ob_{ch}", tag=f"ob_{ch}")
            nc.vector.tensor_copy(oa[:], oa_ps[ch][0:RA, 0:F])
            nc.sync.dma_start(out_d[ch, 2 * KL2:NROWS, :], oa[:])
            nc.vector.tensor_copy(ob[:], ob_ps[ch][0:108, 0:F])
            nc.sync.dma_start(out_d[ch, 0:2 * KL2, :], ob[:])
    nc.finalize()
    return nc


def get_nc():
    global _NC
    if _NC is None:
        _NC = _build_nc()
    return _NC


def _pack_x(xb):
    """(C, T) -> (C, 4, 128, NB0) fp16 with
    xt[ch, rc, r, m] = x[ch, 512*m + 128*rc + r]."""
    xpad = np.zeros((C, NB0 * 512), dtype=np.float32)
    xpad[:, :T] = xb
    return np.ascontiguousarray(
        xpad.reshape(C, NB0, 4, 128).transpose(0, 2, 3, 1)).astype(np.float16)


def make_inputs(xb):
    """Per-core input map for one batch sample xb of shape (C, T)."""
    p = _params()
    return {"xtb": _pack_x(xb), "wb0": p["wb0"], "wrest": p["wrest"]}


def kernel(x):
    global LAST_RESULTS
    x = np.asarray(x, dtype=np.float32)
    assert x.shape == (B, C, T)
    in_maps = [make_inputs(x[b]) for b in range(B)]
    nc = get_nc()
    res = run_bass_kernel_spmd(nc, in_maps, core_ids=list(range(N_CORES)))
    LAST_RESULTS = res
    out = np.empty((B, C, N_BINS, F, 2), dtype=np.float32)
    for b in range(B):
        raw = np.asarray(res.results[b]["out"]).astype(np.float32)
        out[b] = raw.reshape(C, N_BINS, 2, F).transpose(0, 1, 3, 2)
    return out


# revision 37
# speedup vs baseline: 3.4786x; 1.0492x over previous
"""Trainium2 Bass kernel for a CQT (constant-Q transform) nn.Module.

Reference computation (per batch sample b, channel c):
    out[b, c, k, f, 0] = sum_t x[b, c, f*HOP + t] * w_re[k, t]
    out[b, c, k, f, 1] = sum_t x[b, c, f*HOP + t] * w_im[k, t]
where w_re/w_im are Hann-windowed complex exponentials with per-bin ragged
lengths (longest 11340 samples), HOP=512, 84 bins, 409 frames.

Strategy: data-parallel over the batch (1 sample per NeuronCore, 8 cores).
Per core the correlation is factored through a two-level block basis with
pair-packed contractions, cutting tensor-engine moving rows ~8x vs the
direct banded matmul (9316 rows vs 75440):

  1. Stage 1: each 512-sample hop block of x is projected onto a 128-dim
     orthonormal basis B0 = [Bp | Br]: Bp (64) spans the 512-pieces of the
     level-2 basis (exact to 8e-14), Br (64) the residual of the short
     bins' windows.  4 matmuls x 436 moving rows per channel.
  2. Stage 2: coefficients of 4 adjacent blocks are combined into
     2048-block coefficients in basis B2 (SVD of the 2048-aligned segments
     of bins 0..41; 64 components, tail 3e-10).  Because the combiners
     C_j live entirely in the Bp half, adjacent j are packed into the two
     partition halves of a shifted-stacked Y0 copy: 2 matmuls x 432.
  3. Final: bins 42..83 (windows <= 1002 samples, 1-2 hop blocks)
     correlate against Y0 at shifts 0,1 (2 matmuls x 410); bins 0..41
     correlate against Y2 at 6 block shifts, pair-packed via a
     shifted-stacked Y2 copy (3 matmuls x 410), ragged row prefixes
     accumulating in one PSUM bank.

The basis + coefficient payload is ~100 KB vs 7.9 MB of raw windows.
Everything runs in fp16 (measured end-to-end relative error vs the fp32
reference ~5e-4); output is written fp16 and widened on the host.

Scheduling notes (cost-model driven):
  - DMA completion in the cost model is seq-slot (500 ns) + ~1.3 us DGE
    pipeline + transfer, so inputs are spread over all three DMA queues
    (sync/SP, scalar-queue/Act, gpsimd/SWDGE) with the first-needed tiles
    first; wb0 rides as a 128-column header on ch0's x tiles.
  - No activation-engine compute is emitted (a LoadActFuncSet would stall
    the Act queue's DMAs by 1.3 us).
  - PSUM->SBUF casts are split between DVE and the Pool ALU.
  - The final stage is split into column halves so the first half's
    writeback overlaps the second half's matmuls; output rides fp16.
"""

import math
import os as _os
from contextlib import ExitStack

import numpy as np

import concourse.bass as bass
import concourse.mybir as mybir
import concourse.tile as tile
from concourse import bacc
from concourse.bass_utils import run_bass_kernel_spmd

# ---- problem constants (hardcoded CQT spec) ----
SR = 22050
N_BINS = 84
BPO = 12
FMIN = 32.7
HOP = 512
B, C, T = 8, 2, 220500
N_CORES = 8

LMAX = 11340           # longest window
F = 409                # frames: 1 + (T - LMAX)//HOP
FP = 410               # even moving-dim padding for the final stage
FH = 206               # final-stage column split: [0:FH) and [FH:FP)
NB0 = 436              # 512-sample x blocks incl. pad (431 real + shift room)
NB2 = 432              # 2048-block coefficient positions (F + 4*5 + pad)
HDR = 128              # wb0 header columns on ch0 x tiles
KL2 = 42               # bins 0..KL2-1 via 2048-blocks, rest via 512-blocks
Q2 = 64                # level-2 basis size (fits one partition half)
NROWS = 2 * N_BINS
RA = NROWS - 2 * KL2   # 84 rows (bins 42..83) through the level-0 path
RB = 2 * KL2           # 84 rows (bins 0..41) through the level-2 path

TL_DT = mybir.dt.float16

# copy-engine assignment: y0f(0), y0f(1), y2s(0), y2s(1), oa0, oa1, ob0, ob1
# ('v' = DVE, 's' = Activation); tuned by cost-model sweep
K_CFG = _os.environ.get("K_CFG", "vvssvsvs")

_PREP = None
_NC = None
LAST_RESULTS = None


def _params():
    """Host-side constants: bases + projection coefficients (float64 SVD)."""
    global _PREP
    if _PREP is not None:
        return _PREP

    Qf = 1.0 / (2.0 ** (1.0 / BPO) - 1.0)
    freqs = FMIN * 2.0 ** (np.arange(N_BINS, dtype=np.float64) / BPO)
    lengths = np.round(Qf * SR / freqs).astype(np.int64)
    assert int(lengths.max()) == LMAX

    t = np.arange(LMAX, dtype=np.float64)
    L = lengths.astype(np.float64)[:, None]
    mask = (t[None, :] < L).astype(np.float64)
    win = 0.5 * (1.0 - np.cos(2.0 * math.pi * t[None, :] / L)) * mask
    phase = (2.0 * math.pi / SR) * freqs[:, None] * t[None, :]
    W = np.zeros((NROWS, 6 * 2048), dtype=np.float64)
    W[0::2, :LMAX] = win * np.cos(phase)
    W[1::2, :LMAX] = -win * np.sin(phase)

    def seg_matrix(bins, blk):
        out = []
        for k in bins:
            for u in range(math.ceil(int(lengths[k]) / blk)):
                out.append(W[2 * k, blk * u:blk * (u + 1)])
                out.append(W[2 * k + 1, blk * u:blk * (u + 1)])
        return np.array(out)

    # B2: 64-dim basis of the 2048-aligned segments of bins 0..KL2-1
    _, _, V2 = np.linalg.svd(seg_matrix(range(KL2), 2048),
                             full_matrices=False)
    B2 = V2[:Q2]                                    # (64, 2048)
    # Bp: 64-dim basis of B2's 512-pieces (numerically exact)
    pieces = np.concatenate([B2[:, 512 * j:512 * (j + 1)] for j in range(4)],
                            axis=0)
    _, _, Vp = np.linalg.svd(pieces, full_matrices=False)
    Bp = Vp[:64]
    # Br: 64-dim basis of the short bins' segments, residual to Bp
    M0t = seg_matrix(range(KL2, N_BINS), 512)
    resid = M0t - (M0t @ Bp.T) @ Bp
    _, _, Vr = np.linalg.svd(resid, full_matrices=False)
    B0 = np.concatenate([Bp, Vr[:64]], axis=0)      # (128, 512) orthonormal

    # stage-2 combiners, zero-padded to full 128 contraction rows so s2 can
    # read the same full Y0 copy the final stage uses
    Cj = [np.concatenate([Bp @ B2[:, 512 * j:512 * (j + 1)].T,
                          np.zeros((64, Q2))], axis=0)
          for j in range(4)]                        # (128, 64) each
    # final-stage coefficients
    A0 = [B0 @ W[2 * KL2:, 512 * u:512 * (u + 1)].T for u in range(2)]
    rows0 = [RA, 2 * int((np.ceil(lengths[KL2:] / 512.0) > 1).sum())]  # 84,24
    nb2 = np.ceil(lengths[:KL2] / 2048.0).astype(np.int64)
    U2 = int(nb2.max())                             # 6
    rows_u2 = [2 * int((nb2 > u).sum()) for u in range(U2)]
    A2 = [B2 @ W[:rows_u2[u], 2048 * u:2048 * (u + 1)].T for u in range(U2)]
    # f2 pair blocks: [A2_{2t} (q<64); A2_{2t+1} zero-padded (64+q)]
    A2p = []
    for tp in range(U2 // 2):
        r0, r1 = rows_u2[2 * tp], rows_u2[2 * tp + 1]
        blk = np.zeros((128, r0))
        blk[:64] = A2[2 * tp]
        blk[64:, :r1] = A2[2 * tp + 1]
        A2p.append(blk)

    # wb0[r, 128*rc + p] = B0[p, 128*rc + r]  (stationary for stage-1 chunk rc)
    wb0 = np.zeros((128, 512), dtype=np.float16)
    for rc in range(4):
        wb0[:, 128 * rc:128 * (rc + 1)] = B0[:, 128 * rc:128 * (rc + 1)].T
    # wrest = [C_0..C_3 | A0_0 | A0_1 | A2 pairs]
    blocks = Cj + [A0[0], A0[1]] + A2p
    offs = np.cumsum([0] + [b.shape[1] for b in blocks])
    wrest = np.zeros((128, int(offs[-1])), dtype=np.float16)
    for b_, o in zip(blocks, offs[:-1]):
        wrest[:, int(o):int(o) + b_.shape[1]] = b_

    _PREP = dict(wb0=wb0, wrest=wrest, offs=[int(o) for o in offs],
                 rows0=rows0, rows_u2=rows_u2, U2=U2)
    return _PREP


def _build_nc():
    p = _params()
    offs, rows0, rows_u2 = p["offs"], p["rows0"], p["rows_u2"]
    WREST = offs[-1]

    nc = bacc.Bacc(None, target_bir_lowering=False)
    xh_d = nc.dram_tensor("xh", (4, 128, HDR + NB0), TL_DT,
                          kind="ExternalInput")
    x1_d = nc.dram_tensor("x1", (4, 128, NB0), TL_DT, kind="ExternalInput")
    wrest_d = nc.dram_tensor("wrest", (128, WREST), TL_DT,
                             kind="ExternalInput")
    out_d = nc.dram_tensor("out", (C, NROWS, F), TL_DT, kind="ExternalOutput")

    with ExitStack() as ctx:
        tc = ctx.enter_context(tile.TileContext(nc))
        xp = ctx.enter_context(tc.tile_pool(name="xp", bufs=1))
        wp = ctx.enter_context(tc.tile_pool(name="wp", bufs=1))
        yp = ctx.enter_context(tc.tile_pool(name="yp", bufs=1))
        op = ctx.enter_context(tc.tile_pool(name="op", bufs=1))
        pp = ctx.enter_context(tc.tile_pool(name="pp", bufs=1, space="PSUM"))

        # PSUM: 4 banks per channel (Y0, Y2, outA, outB) = all 8 banks
        y0_ps = {ch: pp.tile([128, 512], mybir.dt.float32, name=f"y0p_{ch}",
                             tag=f"y0p_{ch}") for ch in range(C)}
        y2_ps = {ch: pp.tile([128, 512], mybir.dt.float32, name=f"y2p_{ch}",
                             tag=f"y2p_{ch}") for ch in range(C)}
        oa_ps = {ch: pp.tile([128, 512], mybir.dt.float32, name=f"oap_{ch}",
                             tag=f"oap_{ch}") for ch in range(C)}
        ob_ps = {ch: pp.tile([128, 512], mybir.dt.float32, name=f"obp_{ch}",
                             tag=f"obp_{ch}") for ch in range(C)}

        # --- SBUF tiles ---
        xh_sb = {rc: xp.tile([128, HDR + NB0], TL_DT, name=f"xh_{rc}",
                             tag=f"xh_{rc}") for rc in range(4)}
        x1_sb = {rc: xp.tile([128, NB0], TL_DT, name=f"x1_{rc}",
                             tag=f"x1_{rc}") for rc in range(4)}
        wrest_sb = wp.tile([128, WREST], TL_DT, name="wrest_sb",
                           tag="wrest_sb")
        y0f_sb = {ch: yp.tile([128, NB0], TL_DT, name=f"y0f_{ch}",
                              tag=f"y0f_{ch}") for ch in range(C)}
        y2s_sb = {ch: yp.tile([128, NB2], TL_DT, name=f"y2s_{ch}",
                              tag=f"y2s_{ch}") for ch in range(C)}

        # --- input DMA plan (three queues, first-needed tiles first).
        # The scalar engine does PSUM readback copies, so its queue leads
        # with a 1.3us LoadActFuncSet; only the latest-needed x tile rides
        # behind it.  GPSIMD compute cannot touch PSUM (BIR rule), so the
        # Pool queue is input DMAs only.
        nc.sync.dma_start(xh_sb[0][:], xh_d[0])
        nc.sync.dma_start(xh_sb[2][:], xh_d[2])
        nc.sync.dma_start(x1_sb[0][:], x1_d[0])
        nc.sync.dma_start(x1_sb[3][:], x1_d[3])
        nc.gpsimd.dma_start(xh_sb[1][:], xh_d[1])
        nc.gpsimd.dma_start(xh_sb[3][:], xh_d[3])
        nc.gpsimd.dma_start(x1_sb[1][:], x1_d[1])
        nc.gpsimd.dma_start(wrest_sb[:], wrest_d[:])
        nc.scalar.dma_start(x1_sb[2][:], x1_d[2])

        # --- PE stream ---
        def s1(ch):
            for rc in range(4):
                mov = (xh_sb[rc][:, HDR:HDR + NB0] if ch == 0
                       else x1_sb[rc][:, 0:NB0])
                nc.tensor.matmul(y0_ps[ch][0:128, 0:NB0],
                                 xh_sb[rc][:, 0:HDR], mov,
                                 start=(rc == 0), stop=(rc == 3),
                                 skip_group_check=True)

        SH = 216  # stage-2 column split

        def s2h(ch, c0, c1):
            for j in range(4):
                nc.tensor.matmul(y2_ps[ch][0:Q2, c0:c1],
                                 wrest_sb[:, 64 * j:64 * (j + 1)],
                                 y0f_sb[ch][:, c0 + j:c1 + j],
                                 start=(j == 0), stop=(j == 3),
                                 skip_group_check=True)

        def f0(ch):
            for u in range(2):
                nc.tensor.matmul(oa_ps[ch][0:rows0[u], 0:FP],
                                 wrest_sb[:, offs[4 + u]:offs[4 + u]
                                          + rows0[u]],
                                 y0f_sb[ch][:, u:u + FP],
                                 start=(u == 0), stop=(u == 1),
                                 skip_group_check=True)

        def f2(ch, ps, c0, c1):
            for tp in range(3):
                m = rows_u2[2 * tp]
                off = offs[6 + tp]
                nc.tensor.matmul(ps[0:m, 0:c1 - c0],
                                 wrest_sb[:, off:off + m],
                                 y2s_sb[ch][:, 8 * tp + c0:8 * tp + c1],
                                 start=(tp == 0), stop=(tp == 2),
                                 skip_group_check=True)

        # --- PSUM -> SBUF casts, balanced across the two PSUM-capable
        # engines (DVE and Activation; GPSIMD may not touch PSUM) ---
        def _cp(i):
            return (nc.vector.tensor_copy if K_CFG[i] == "v"
                    else nc.scalar.copy)

        def copy_y0(ch):
            _cp(ch)(y0f_sb[ch][:, 0:SH + 4], y0_ps[ch][:, 0:SH + 4])
            _cp(ch)(y0f_sb[ch][:, SH + 4:NB0], y0_ps[ch][:, SH + 4:NB0])

        def copy_y2(ch):
            # stacked-shift copies: rows 64+q hold Y2[q, m+4] for the f2
            # pair-packing
            eng = _cp(2 + ch)
            eng(y2s_sb[ch][0:64, 0:NB2], y2_ps[ch][0:64, 0:NB2])
            eng(y2s_sb[ch][64:128, 0:NB2 - 4], y2_ps[ch][0:64, 4:NB2])

        FB = 272  # f2(1) column split: [0:FB) -> ob_ps[1], rest -> y2_ps[0]

        def out_a(ch, queue):
            o = op.tile([RA, F], TL_DT, name=f"oa{ch}", tag=f"oa{ch}")
            _cp(4 + ch)(o[:], oa_ps[ch][0:RA, 0:F])
            queue.dma_start(out_d[ch, RB:NROWS, :], o[:])

        def out_b(ch, queue):
            o = op.tile([RB, F], TL_DT, name=f"ob{ch}", tag=f"ob{ch}")
            _cp(6 + ch)(o[:], ob_ps[ch][0:RB, 0:F])
            queue.dma_start(out_d[ch, 0:RB, :], o[:])

        s1(0)
        copy_y0(0)
        s1(1)
        copy_y0(1)
        s2h(0, 0, SH)
        s2h(0, SH, NB2)
        copy_y2(0)
        s2h(1, 0, SH)
        s2h(1, SH, NB2)
        copy_y2(1)
        f0(0)
        out_a(0, nc.sync)
        f0(1)
        out_a(1, nc.gpsimd)
        f2(0, ob_ps[0], 0, FP)
        out_b(0, nc.sync)
        f2(1, ob_ps[1], 0, FP)
        out_b(1, nc.scalar)
    nc.finalize()
    return nc


def get_nc():
    global _NC
    if _NC is None:
        _NC = _build_nc()
    return _NC


def _pack_x(xb):
    """(C, T) -> (C, 4, 128, NB0) fp16 with
    xt[ch, rc, r, m] = x[ch, 512*m + 128*rc + r]."""
    xpad = np.zeros((C, NB0 * 512), dtype=np.float32)
    xpad[:, :T] = xb
    return np.ascontiguousarray(
        xpad.reshape(C, NB0, 4, 128).transpose(0, 2, 3, 1)).astype(np.float16)


def make_inputs(xb):
    """Per-core input map for one batch sample xb of shape (C, T)."""
    p = _params()
    xt = _pack_x(xb)
    xh = np.zeros((4, 128, HDR + NB0), dtype=np.float16)
    for rc in range(4):
        xh[rc, :, :HDR] = p["wb0"][:, 128 * rc:128 * (rc + 1)]
        xh[rc, :, HDR:] = xt[0, rc]
    return {"xh": xh, "x1": xt[1], "wrest": p["wrest"]}


def kernel(x):
    global LAST_RESULTS
    x = np.asarray(x, dtype=np.float32)
    assert x.shape == (B, C, T)
    in_maps = [make_inputs(x[b]) for b in range(B)]
    nc = get_nc()
    res = run_bass_kernel_spmd(nc, in_maps, core_ids=list(range(N_CORES)))
    LAST_RESULTS = res
    out = np.empty((B, C, N_BINS, F, 2), dtype=np.float32)
    for b in range(B):
        raw = np.asarray(res.results[b]["out"]).astype(np.float32)
        out[b] = raw.reshape(C, N_BINS, 2, F).transpose(0, 1, 3, 2)
    return out
